# revision 1
# baseline (speedup 1.0000x reference)
"""DepthwiseXCorr (SiamRPN-style depthwise cross-correlation head) on 8 trn2 cores.

Data-parallel over batch: B=128 -> 16 samples per core. Per sample:
  branch(x) = BN2(pw1x1(ReLU6(BN1(dw3x3(x)))))   for kernel (7x7) and search (31x31)
  out = per-channel xcorr(search_feat 29x29, kernel_feat 5x5) -> 25x25

On-chip mapping (per core):
  - channels on partitions, 2 blocks of 128 (C=256)
  - dw conv: 9 shifted-window FMAs on VectorE (scalar_tensor_tensor, per-partition
    weight scalar), BN1 folded into weights host-side, bias fused into tap 0
  - ReLU6: one tensor_scalar (min 6, max 0)
  - pw conv: PE matmuls (float32r), BN2 scale folded into weights host-side,
    bias added by ScalarE Identity-activation while evicting PSUM
  - xcorr: 25 shifted-window FMAs on VectorE with kernel-feat values as
    per-partition scalars
"""

import numpy as np

import concourse.bass as bass
import concourse.mybir as mybir
from concourse.tile import TileContext
from concourse.bass_utils import run_bass_kernel_spmd

F32 = mybir.dt.float32
F32R = mybir.dt.float32r
AF = mybir.ActivationFunctionType
OP = mybir.AluOpType

B, C, KH, SH, KK = 128, 256, 7, 31, 3
N_CORES = 8
BPC = B // N_CORES          # samples per core
G = C // 128                # channel blocks
EPS = 1e-5

# packed params column offsets: [wdk 18 | bdk 2 | wds 18 | bds 2 | bpk 2 | bps 2 |
#                                wpk 512 | wps 512]
O_WDK, O_BDK, O_WDS, O_BDS = 0, 18, 20, 38
O_BPK, O_BPS, O_WPK, O_WPS = 40, 42, 44, 44 + 512
P_TOT = 44 + 1024

_cache: dict = {}

LAST_RESULTS = None         # stash for test harness (exec_time_ns etc.)


def _fold_branch(dw_w, bn1, pw_w, pw_b, bn2):
    """Fold eval-mode BN params into conv weights/biases (host, numpy fp32)."""
    g1, b1, m1, v1 = bn1[0], bn1[1], bn1[2], bn1[3]
    inv1 = g1 / np.sqrt(v1 + EPS)
    shift1 = b1 - m1 * inv1
    dw = (dw_w[:, 0] * inv1[:, None, None]).reshape(C, 9).astype(np.float32)

    g2, b2, m2, v2 = bn2[0], bn2[1], bn2[2], bn2[3]
    inv2 = g2 / np.sqrt(v2 + EPS)
    shift2 = b2 - m2 * inv2
    W = (pw_w[:, :, 0, 0] * inv2[:, None]).astype(np.float32)   # (co, ci)
    bias2 = (pw_b * inv2 + shift2).astype(np.float32)

    # lhsT blocks for PE: lhsT[gi, go][ci_in, co_in] = W[go*128+co_in, gi*128+ci_in]
    lhsT = np.zeros((G, G, 128, 128), np.float32)
    for gi in range(G):
        for go in range(G):
            lhsT[gi, go] = W[go * 128:(go + 1) * 128, gi * 128:(gi + 1) * 128].T
    dw_blk = dw.reshape(G, 128, 9)
    b1_blk = shift1.astype(np.float32).reshape(G, 128, 1)
    b2_blk = bias2.reshape(G, 128, 1)
    return dw_blk, b1_blk, lhsT, b2_blk


def _split_waits(nc, keep=1):
    """This container's walrus accepts only one sync-wait per instruction.
    Move extra waits onto standalone EventSemaphore instructions placed just
    before the owning instruction in its engine stream (same semantics: the
    engine's sequencer stalls on each in turn)."""
    import bass_rust

    n = 0
    for bb in nc.m.functions[0].blocks:
        out = []
        for ins in bb.instructions:
            si = ins.sync_info
            if si is not None and len(si.on_wait) > keep:
                waits = list(si.on_wait)
                for w in waits[:-keep]:
                    n += 1
                    ev = mybir.InstEventSemaphore(
                        name=f"antsplitw_{n}", ins=[], outs=[])
                    ev.engine = ins.engine
                    ev.sync_info = bass_rust.SyncInfo(on_wait=[w], on_update=[])
                    out.append(ev)
                ins.sync_info = bass_rust.SyncInfo(
                    on_wait=waits[-keep:], on_update=list(si.on_update))
            out.append(ins)
        bb.instructions = out
    return n


def _build_nc():
    """Build the per-core Bass kernel (same program on all 8 cores)."""
    nc = bass.Bass()

    kern_h = nc.declare_dram_parameter("kern_in", [BPC, C, KH, KH], F32, isOutput=False)
    srch_h = nc.declare_dram_parameter("srch_in", [BPC, C, SH, SH], F32, isOutput=False)
    prm_h = nc.declare_dram_parameter("params", [128, P_TOT], F32, isOutput=False)
    out_h = nc.declare_dram_parameter("out", [BPC, C, 25, 25], F32, isOutput=True)

    HO_K, HO_S, HO_X = KH - 2, SH - 2, 25   # 5, 29, 25

    with TileContext(nc) as tc:
        with (
            tc.tile_pool(name="const", bufs=1) as cpool,
            tc.tile_pool(name="kio", bufs=3) as kpool,
            tc.tile_pool(name="sio", bufs=2) as spool,
            tc.tile_pool(name="feat", bufs=4) as fpool,
            tc.tile_pool(name="xout", bufs=3) as xpool,
            tc.tile_pool(name="ps", bufs=2, space="PSUM") as ppool,
        )        :
            # ---- constants into SBUF (single packed DMA) ----
            prm_sb = cpool.tile([128, P_TOT], F32)
            nc.sync.dma_start(out=prm_sb[:], in_=prm_h[:])

            def _wd(base, g, t):      # dw weight col [128,1]
                return prm_sb[:, base + g * 9 + t:base + g * 9 + t + 1]

            def _b(base, g):          # bias col [128,1]
                return prm_sb[:, base + g:base + g + 1]

            def _wp(base, gi, go):    # pw lhsT block [128,128]
                o = base + (gi * G + go) * 128
                return prm_sb[:, o:o + 128]

            def dwconv_relu6(x, wbase, bbase, g, hi, ho, tag):
                """9-tap depthwise conv + bias + relu6; returns [128, ho*ho] tile."""
                acc = fpool.tile([128, ho, ho], F32, tag=f"acc_{tag}", name=f"acc_{tag}")
                nc.vector.tensor_scalar(
                    acc[:], x[:, 0:ho, 0:ho], _wd(wbase, g, 0), _b(bbase, g),
                    OP.mult, OP.add)
                for t in range(1, 9):
                    u, v = t // 3, t % 3
                    nc.vector.scalar_tensor_tensor(
                        acc[:], x[:, u:u + ho, v:v + ho], _wd(wbase, g, t),
                        acc[:], OP.mult, OP.add)
                h = fpool.tile([128, ho * ho], F32, tag=f"h_{tag}", name=f"h_{tag}")
                nc.vector.tensor_scalar(
                    h[:], acc[:].rearrange("p a b -> p (a b)"), 6.0, 0.0,
                    OP.min, OP.max)
                return h

            for b in range(BPC):
                # ---- kernel branch ----
                hk = []
                for g in range(G):
                    xk = kpool.tile([128, KH, KH], F32, name="xk")
                    nc.sync.dma_start(out=xk[:], in_=kern_h[b, 128 * g:128 * (g + 1)])
                    hk.append(dwconv_relu6(xk, O_WDK, O_BDK, g, KH, HO_K, "k"))
                K2 = []
                for go in range(G):
                    psk = ppool.tile([128, HO_K * HO_K], F32, name="psk")
                    for gi in range(G):
                        nc.tensor.matmul(
                            psk[:], _wp(O_WPK, gi, go),
                            hk[gi][:],
                            start=(gi == 0), stop=(gi == G - 1))
                    k2 = fpool.tile([128, HO_K * HO_K], F32, name="k2")
                    nc.scalar.activation(k2[:], psk[:], AF.Identity,
                                         bias=_b(O_BPK, go), scale=1.0)
                    K2.append(k2)

                # ---- search branch ----
                hs = []
                for g in range(G):
                    xs = spool.tile([128, SH, SH], F32, name="xs")
                    nc.sync.dma_start(out=xs[:], in_=srch_h[b, 128 * g:128 * (g + 1)])
                    hs.append(dwconv_relu6(xs, O_WDS, O_BDS, g, SH, HO_S, "s"))
                S2 = []
                NS = HO_S * HO_S  # 841
                for go in range(G):
                    s2 = fpool.tile([128, HO_S, HO_S], F32, name="s2")
                    s2f = s2[:].rearrange("p a b -> p (a b)")
                    for (n0, n1) in ((0, 512), (512, NS)):
                        pss = ppool.tile([128, 512], F32, name="pss")
                        for gi in range(G):
                            nc.tensor.matmul(
                                pss[:, 0:n1 - n0],
                                _wp(O_WPS, gi, go),
                                hs[gi][:, n0:n1],
                                start=(gi == 0), stop=(gi == G - 1))
                        nc.scalar.activation(s2f[:, n0:n1], pss[:, 0:n1 - n0],
                                             AF.Identity, bias=_b(O_BPS, go),
                                             scale=1.0)
                    S2.append(s2)

                # ---- depthwise xcorr ----
                for g in range(G):
                    accx = xpool.tile([128, 25, 25], F32, name="accx")
                    nc.vector.tensor_scalar(
                        accx[:], S2[g][:, 0:25, 0:25], K2[g][:, 0:1], None, OP.mult)
                    for t in range(1, 25):
                        u, v = t // 5, t % 5
                        nc.vector.scalar_tensor_tensor(
                            accx[:], S2[g][:, u:u + 25, v:v + 25],
                            K2[g][:, t:t + 1], accx[:], OP.mult, OP.add)
                    nc.sync.dma_start(out=out_h[b, 128 * g:128 * (g + 1)],
                                      in_=accx[:])
    _split_waits(nc)
    return nc


def kernel(kernel, search, k_dw_w, k_bn1, k_pw_w, k_pw_b, k_bn2,
           s_dw_w, s_bn1, s_pw_w, s_pw_b, s_bn2):
    global LAST_RESULTS
    kdw, kb1, kpw, kb2 = _fold_branch(np.asarray(k_dw_w), np.asarray(k_bn1),
                                      np.asarray(k_pw_w), np.asarray(k_pw_b),
                                      np.asarray(k_bn2))
    sdw, sb1, spw, sb2 = _fold_branch(np.asarray(s_dw_w), np.asarray(s_bn1),
                                      np.asarray(s_pw_w), np.asarray(s_pw_b),
                                      np.asarray(s_bn2))
    kern = np.ascontiguousarray(np.asarray(kernel, np.float32))
    srch = np.ascontiguousarray(np.asarray(search, np.float32))

    if "nc" not in _cache:
        _cache["nc"] = _build_nc()
    nc = _cache["nc"]

    prm = np.zeros((128, P_TOT), np.float32)
    prm[:, O_WDK:O_WDK + 18] = kdw.transpose(1, 0, 2).reshape(128, 18)
    prm[:, O_BDK:O_BDK + G] = kb1.transpose(1, 0, 2).reshape(128, G)
    prm[:, O_WDS:O_WDS + 18] = sdw.transpose(1, 0, 2).reshape(128, 18)
    prm[:, O_BDS:O_BDS + G] = sb1.transpose(1, 0, 2).reshape(128, G)
    prm[:, O_BPK:O_BPK + G] = kb2.transpose(1, 0, 2).reshape(128, G)
    prm[:, O_BPS:O_BPS + G] = sb2.transpose(1, 0, 2).reshape(128, G)
    prm[:, O_WPK:O_WPK + 512] = kpw.transpose(2, 0, 1, 3).reshape(128, 512)
    prm[:, O_WPS:O_WPS + 512] = spw.transpose(2, 0, 1, 3).reshape(128, 512)

    in_maps = []
    for i in range(N_CORES):
        sl = slice(i * BPC, (i + 1) * BPC)
        in_maps.append({"kern_in": kern[sl], "srch_in": srch[sl], "params": prm})

    res = run_bass_kernel_spmd(nc, in_maps, list(range(N_CORES)))
    LAST_RESULTS = res
    out = np.concatenate([res.results[i]["out"] for i in range(N_CORES)], axis=0)
    return out



# revision 3
# speedup vs baseline: 1.4968x; 1.4968x over previous
"""DepthwiseXCorr (SiamRPN-style depthwise cross-correlation head) on 8 trn2 cores.

Data-parallel over batch: B=128 -> 16 samples per core. Per sample:
  branch(x) = BN2(pw1x1(ReLU6(BN1(dw3x3(x)))))   for kernel (7x7) and search (31x31)
  out = per-channel xcorr(search_feat 29x29, kernel_feat 5x5) -> 25x25

Multi-engine mapping (per core), replacing the DVE-bound baseline:
  - PE: ALL convs as matmuls.  Depthwise conv = 9 accumulating matmuls with
    host-precomputed diagonal weight matrices (fp16, FWL fast loads).
    Pointwise 1x1 = regular matmuls (BN2 scale folded host-side).
    xcorr: P_TAPS of the 25 taps as diag matmuls whose diagonals are built
    on-chip from the kernel features.
  - ACT (ScalarE): every PSUM eviction fused with BN bias (+ReLU for dw),
    and the xcorr diag builds (Copy activation, per-partition scale=k2 col).
  - DVE: remaining xcorr taps as fp32 scalar_tensor_tensor; the first DVE
    tap reads the PE's PSUM partial directly as its accumulator seed.
  - GPSIMD: the ReLU6 min-6 clamps (1x-mode DVE never contends for the
    shared SBUF port pair).
Loop is software-pipelined: PE order per sample b is
  dw(b) -> xcorr(b-1) -> pw(b), hiding all eviction latencies.
"""

import numpy as np

import concourse.bass as bass
import concourse.mybir as mybir
from concourse.tile import TileContext
from concourse.bass_utils import run_bass_kernel_spmd

F32 = mybir.dt.float32
F16 = mybir.dt.float16
AF = mybir.ActivationFunctionType
OP = mybir.AluOpType

B, C, KH, SH, KK = 128, 256, 7, 31, 3
N_CORES = 8
BPC = B // N_CORES          # samples per core
G = C // 128                # channel blocks
EPS = 1e-5

HO_K, HO_S, HO_X = KH - 2, SH - 2, 25   # 5, 29, 25

# --- tunables -------------------------------------------------------------
P_TAPS = 15                 # xcorr taps done on PE (of 25); rest on DVE
# row split of the 25x25 xcorr output into two PSUM banks
XR_A = 13                   # rows 0..12  -> 325 cols
XR_B = HO_X - XR_A          # rows 13..24 -> 300 cols
# row split of the 29x29 dw/pw output into two PSUM banks
SR_A = 17                   # rows 0..16  -> 493 cols
SR_B = HO_S - SR_A          # rows 17..28 -> 348 cols

TAPS = [(t // 5, t % 5) for t in range(25)]
PE_TAPS = TAPS[:P_TAPS]
DVE_TAPS = TAPS[P_TAPS:]

# fp16 params column offsets
O_DWK = 0                       # 2g * 9taps * 128
O_DWS = O_DWK + G * 9 * 128
O_PWK = O_DWS + G * 9 * 128     # 4 blocks * 128
O_PWS = O_PWK + G * G * 128
O_I = O_PWS + G * G * 128       # identity 128
NP16 = O_I + 128
# fp32 params column offsets (biases)
O_BDK, O_BDS, O_BPK, O_BPS = 0, G, 2 * G, 3 * G
NP32 = 4 * G

_cache: dict = {}

LAST_RESULTS = None         # stash for test harness (exec_time_ns etc.)


def _fold_branch(dw_w, bn1, pw_w, pw_b, bn2):
    """Fold eval-mode BN params into conv weights/biases (host, numpy fp32)."""
    g1, b1, m1, v1 = bn1[0], bn1[1], bn1[2], bn1[3]
    inv1 = g1 / np.sqrt(v1 + EPS)
    shift1 = (b1 - m1 * inv1).astype(np.float32)
    dw = (dw_w[:, 0] * inv1[:, None, None]).reshape(C, 9).astype(np.float32)

    g2, b2, m2, v2 = bn2[0], bn2[1], bn2[2], bn2[3]
    inv2 = g2 / np.sqrt(v2 + EPS)
    shift2 = b2 - m2 * inv2
    W = (pw_w[:, :, 0, 0] * inv2[:, None]).astype(np.float32)   # (co, ci)
    bias2 = (pw_b * inv2 + shift2).astype(np.float32)

    # dw diag blocks: ddiag[g, t] = diag(dw[128g:128(g+1), t])  [ci, co]
    ddiag = np.zeros((G, 9, 128, 128), np.float32)
    for g in range(G):
        for t in range(9):
            np.fill_diagonal(ddiag[g, t], dw[128 * g:128 * (g + 1), t])

    # pw lhsT blocks for PE: lhsT[gi, go][ci, co] = W[go*128+co, gi*128+ci]
    lhsT = np.zeros((G, G, 128, 128), np.float32)
    for gi in range(G):
        for go in range(G):
            lhsT[gi, go] = W[go * 128:(go + 1) * 128, gi * 128:(gi + 1) * 128].T
    b1_blk = shift1.reshape(G, 128)
    b2_blk = bias2.reshape(G, 128)
    return ddiag, b1_blk, lhsT, b2_blk


def _split_waits(nc, keep=1):
    """This container's walrus accepts only one sync-wait per instruction.
    Move extra waits onto standalone EventSemaphore instructions placed just
    before the owning instruction in its engine stream (same semantics: the
    engine's sequencer stalls on each in turn)."""
    import bass_rust

    n = 0
    for bb in nc.m.functions[0].blocks:
        out = []
        for ins in bb.instructions:
            si = ins.sync_info
            if si is not None and len(si.on_wait) > keep:
                waits = list(si.on_wait)
                for w in waits[:-keep]:
                    n += 1
                    ev = mybir.InstEventSemaphore(
                        name=f"antsplitw_{n}", ins=[], outs=[])
                    ev.engine = ins.engine
                    ev.sync_info = bass_rust.SyncInfo(on_wait=[w], on_update=[])
                    out.append(ev)
                ins.sync_info = bass_rust.SyncInfo(
                    on_wait=waits[-keep:], on_update=list(si.on_update))
            out.append(ins)
        bb.instructions = out
    return n


def _build_nc():
    """Build the per-core Bass kernel (same program on all 8 cores)."""
    nc = bass.Bass()

    kern_h = nc.declare_dram_parameter("kern_in", [BPC, C, KH, KH], F16, isOutput=False)
    srch_h = nc.declare_dram_parameter("srch_in", [BPC, C, SH, SH], F16, isOutput=False)
    p16_h = nc.declare_dram_parameter("prm16", [128, NP16], F16, isOutput=False)
    p32_h = nc.declare_dram_parameter("prm32", [128, NP32], F32, isOutput=False)
    out_h = nc.declare_dram_parameter("out", [BPC, C, HO_X, HO_X], F32, isOutput=True)

    with TileContext(nc) as tc:
        with (
            tc.tile_pool(name="const", bufs=1) as cpool,
            tc.tile_pool(name="sio", bufs=3) as spool,
            tc.tile_pool(name="hbuf", bufs=2) as hpool,
            tc.tile_pool(name="s2buf", bufs=2) as s2pool,
            tc.tile_pool(name="diag", bufs=2) as dpool,
            tc.tile_pool(name="xout", bufs=2) as xpool,
            tc.tile_pool(name="ps", bufs=2, space="PSUM") as ppool,
        ):
            # ---- constants into SBUF ----
            p16 = cpool.tile([128, NP16], F16)
            nc.sync.dma_start(out=p16[:], in_=p16_h[:])
            p32 = cpool.tile([128, NP32], F32)
            nc.sync.dma_start(out=p32[:], in_=p32_h[:])

            def _dwd(base, g, t):     # dw diag lhsT [128,128] fp16
                o = base + (g * 9 + t) * 128
                return p16[:, o:o + 128]

            def _wp(base, gi, go):    # pw lhsT block [128,128] fp16
                o = base + (gi * G + go) * 128
                return p16[:, o:o + 128]

            def _eye():               # identity [128,128] fp16
                return p16[:, O_I:O_I + 128]

            def _b(base, g):          # bias col [128,1] fp32
                return p32[:, base + g:base + g + 1]

            # =========== Phase A: kernel branch, all 16 samples ===========
            hk = []
            for g in range(G):
                xk = cpool.tile([128, BPC, KH, KH], F16, name=f"xk{g}")
                for b in range(BPC):
                    nc.sync.dma_start(out=xk[:, b],
                                      in_=kern_h[b, 128 * g:128 * (g + 1)])
                psK = ppool.tile([128, BPC, HO_K, HO_K], F32, tag="psa",
                                 name="psK")
                for t in range(9):
                    u, v = t // 3, t % 3
                    nc.tensor.matmul(
                        psK[:], _dwd(O_DWK, g, t),
                        xk[:, :, u:u + HO_K, v:v + HO_K],
                        start=(t == 0), stop=(t == 8))
                h = cpool.tile([128, BPC, HO_K, HO_K], F16, name=f"hk{g}")
                nc.scalar.activation(h[:], psK[:], AF.Relu, bias=_b(O_BDK, g),
                                     scale=1.0)
                nc.gpsimd.tensor_scalar(h[:], h[:], 6.0, None, OP.min)
                hk.append(h)
            K2 = []
            for go in range(G):
                psK2 = ppool.tile([128, BPC, HO_K, HO_K], F32, tag="psb",
                                  name="psK2")
                for gi in range(G):
                    nc.tensor.matmul(psK2[:], _wp(O_PWK, gi, go), hk[gi][:],
                                     start=(gi == 0), stop=(gi == G - 1))
                k2 = cpool.tile([128, BPC, HO_K * HO_K], F32, name=f"k2{go}")
                nc.scalar.activation(k2[:], psK2[:], AF.Identity,
                                     bias=_b(O_BPK, go), scale=1.0)
                K2.append(k2)

            # =========== Phase B: search branch + xcorr, pipelined ===========
            def load_xs(b):
                tiles = []
                for g in range(G):
                    xs = spool.tile([128, SH, SH], F16, tag=f"xs{g}",
                                    name=f"xs{g}")
                    nc.sync.dma_start(out=xs[:],
                                      in_=srch_h[b, 128 * g:128 * (g + 1)])
                    tiles.append(xs)
                return tiles

            def dw_search(b, xs_tiles):
                hs = []
                for g in range(G):
                    psa = ppool.tile([128, SR_A, HO_S], F32, tag="psa",
                                     name="psa")
                    psb = ppool.tile([128, SR_B, HO_S], F32, tag="psb",
                                     name="psb")
                    xs = xs_tiles[g]
                    for t in range(9):
                        u, v = t // 3, t % 3
                        nc.tensor.matmul(
                            psa[:], _dwd(O_DWS, g, t),
                            xs[:, u:u + SR_A, v:v + HO_S],
                            start=(t == 0), stop=(t == 8))
                        nc.tensor.matmul(
                            psb[:], _dwd(O_DWS, g, t),
                            xs[:, u + SR_A:u + HO_S, v:v + HO_S],
                            start=(t == 0), stop=(t == 8))
                    h = hpool.tile([128, HO_S * HO_S], F16, tag=f"hs{g}",
                                   name=f"hs{g}")
                    na = SR_A * HO_S
                    nc.scalar.activation(h[:, 0:na], psa[:], AF.Relu,
                                         bias=_b(O_BDS, g), scale=1.0)
                    nc.scalar.activation(h[:, na:HO_S * HO_S], psb[:], AF.Relu,
                                         bias=_b(O_BDS, g), scale=1.0)
                    nc.gpsimd.tensor_scalar(h[:], h[:], 6.0, None, OP.min)
                    hs.append(h)
                return hs

            def pw_search(b, hs):
                s2 = []
                na = SR_A * HO_S
                for go in range(G):
                    psa = ppool.tile([128, SR_A, HO_S], F32, tag="psa",
                                     name="ppa")
                    psb = ppool.tile([128, SR_B, HO_S], F32, tag="psb",
                                     name="ppb")
                    for gi in range(G):
                        nc.tensor.matmul(psa[:], _wp(O_PWS, gi, go),
                                         hs[gi][:, 0:na],
                                         start=(gi == 0), stop=(gi == G - 1))
                        nc.tensor.matmul(psb[:], _wp(O_PWS, gi, go),
                                         hs[gi][:, na:HO_S * HO_S],
                                         start=(gi == 0), stop=(gi == G - 1))
                    t = s2pool.tile([128, HO_S, HO_S], F16, tag=f"s2{go}",
                                    name=f"s2{go}")
                    nc.scalar.activation(t[:, 0:SR_A, :], psa[:], AF.Identity,
                                         bias=_b(O_BPS, go), scale=1.0)
                    nc.scalar.activation(t[:, SR_A:HO_S, :], psb[:],
                                         AF.Identity, bias=_b(O_BPS, go),
                                         scale=1.0)
                    s2.append(t)
                return s2

            def xcorr(b, s2):
                for g in range(G):
                    k2 = K2[g]
                    s2g = s2[g]
                    # ACT: build the P_TAPS diagonal matrices for this unit
                    dall = dpool.tile([128, P_TAPS * 128], F16, tag=f"da{g}",
                                      name=f"da{g}")
                    for i, (u, v) in enumerate(PE_TAPS):
                        ti = u * 5 + v
                        nc.scalar.activation(
                            dall[:, i * 128:(i + 1) * 128], _eye(), AF.Copy,
                            bias=0.0, scale=k2[:, b, ti:ti + 1])
                    # PE: accumulate P_TAPS taps into two PSUM banks
                    pxa = ppool.tile([128, XR_A, HO_X], F32, tag="pxa",
                                     name="pxa")
                    pxb = ppool.tile([128, XR_B, HO_X], F32, tag="pxb",
                                     name="pxb")
                    n = len(PE_TAPS)
                    for i, (u, v) in enumerate(PE_TAPS):
                        d = dall[:, i * 128:(i + 1) * 128]
                        nc.tensor.matmul(
                            pxa[:], d, s2g[:, u:u + XR_A, v:v + HO_X],
                            start=(i == 0), stop=(i == n - 1))
                        nc.tensor.matmul(
                            pxb[:], d,
                            s2g[:, u + XR_A:u + HO_X, v:v + HO_X],
                            start=(i == 0), stop=(i == n - 1))
                    # DVE: remaining taps; first tap seeds from PSUM
                    acc = xpool.tile([128, HO_X, HO_X], F32, tag=f"ax{g}",
                                     name=f"ax{g}")
                    (u0, v0) = DVE_TAPS[0]
                    t0 = u0 * 5 + v0
                    nc.vector.scalar_tensor_tensor(
                        acc[:, 0:XR_A, :],
                        s2g[:, u0:u0 + XR_A, v0:v0 + HO_X],
                        k2[:, b, t0:t0 + 1], pxa[:], OP.mult, OP.add)
                    nc.vector.scalar_tensor_tensor(
                        acc[:, XR_A:HO_X, :],
                        s2g[:, u0 + XR_A:u0 + HO_X, v0:v0 + HO_X],
                        k2[:, b, t0:t0 + 1], pxb[:], OP.mult, OP.add)
                    for (u, v) in DVE_TAPS[1:]:
                        ti = u * 5 + v
                        nc.vector.scalar_tensor_tensor(
                            acc[:], s2g[:, u:u + HO_X, v:v + HO_X],
                            k2[:, b, ti:ti + 1], acc[:], OP.mult, OP.add)
                    nc.sync.dma_start(out=out_h[b, 128 * g:128 * (g + 1)],
                                      in_=acc[:])

            xs_cur = load_xs(0)
            prev_s2 = None
            for b in range(BPC):
                xs_next = load_xs(b + 1) if b + 1 < BPC else None
                hs = dw_search(b, xs_cur)
                if prev_s2 is not None:
                    xcorr(b - 1, prev_s2)
                prev_s2 = pw_search(b, hs)
                xs_cur = xs_next
            xcorr(BPC - 1, prev_s2)

    _split_waits(nc)
    return nc


def kernel(kernel, search, k_dw_w, k_bn1, k_pw_w, k_pw_b, k_bn2,
           s_dw_w, s_bn1, s_pw_w, s_pw_b, s_bn2):
    global LAST_RESULTS
    kdd, kb1, kpw, kb2 = _fold_branch(np.asarray(k_dw_w), np.asarray(k_bn1),
                                      np.asarray(k_pw_w), np.asarray(k_pw_b),
                                      np.asarray(k_bn2))
    sdd, sb1, spw, sb2 = _fold_branch(np.asarray(s_dw_w), np.asarray(s_bn1),
                                      np.asarray(s_pw_w), np.asarray(s_pw_b),
                                      np.asarray(s_bn2))
    kern = np.ascontiguousarray(np.asarray(kernel, np.float16))
    srch = np.ascontiguousarray(np.asarray(search, np.float16))

    if "nc" not in _cache:
        _cache["nc"] = _build_nc()
    nc = _cache["nc"]

    prm16 = np.zeros((128, NP16), np.float16)
    # dw diags [g, t, ci, co] -> [ci, (g,t,co)]
    prm16[:, O_DWK:O_DWK + G * 9 * 128] = \
        kdd.transpose(2, 0, 1, 3).reshape(128, G * 9 * 128).astype(np.float16)
    prm16[:, O_DWS:O_DWS + G * 9 * 128] = \
        sdd.transpose(2, 0, 1, 3).reshape(128, G * 9 * 128).astype(np.float16)
    prm16[:, O_PWK:O_PWK + G * G * 128] = \
        kpw.transpose(2, 0, 1, 3).reshape(128, G * G * 128).astype(np.float16)
    prm16[:, O_PWS:O_PWS + G * G * 128] = \
        spw.transpose(2, 0, 1, 3).reshape(128, G * G * 128).astype(np.float16)
    prm16[:, O_I:O_I + 128] = np.eye(128, dtype=np.float16)

    prm32 = np.zeros((128, NP32), np.float32)
    prm32[:, O_BDK:O_BDK + G] = kb1.T
    prm32[:, O_BDS:O_BDS + G] = sb1.T
    prm32[:, O_BPK:O_BPK + G] = kb2.T
    prm32[:, O_BPS:O_BPS + G] = sb2.T

    in_maps = []
    for i in range(N_CORES):
        sl = slice(i * BPC, (i + 1) * BPC)
        in_maps.append({"kern_in": kern[sl], "srch_in": srch[sl],
                        "prm16": prm16, "prm32": prm32})

    res = run_bass_kernel_spmd(nc, in_maps, list(range(N_CORES)))
    LAST_RESULTS = res
    out = np.concatenate([res.results[i]["out"] for i in range(N_CORES)], axis=0)
    return out


# revision 18
# speedup vs baseline: 2.9586x; 1.9765x over previous
"""DepthwiseXCorr (SiamRPN-style depthwise cross-correlation head) on 8 trn2 cores.

Data-parallel over batch: B=128 -> 16 samples per core. Per sample:
  branch(x) = BN2(pw1x1(ReLU6(BN1(dw3x3(x)))))   for kernel (7x7) and search (31x31)
  out = per-channel xcorr(search_feat 29x29, kernel_feat 5x5) -> 25x25

Multi-engine mapping (per core), replacing the DVE-bound baseline:
  - PE: ALL convs as matmuls.  Depthwise conv = 9 accumulating matmuls with
    host-precomputed diagonal weight matrices (fp16, FWL fast loads).
    Pointwise 1x1 = regular matmuls (BN2 scale folded host-side).
    xcorr: P_TAPS of the 25 taps as diag matmuls whose diagonals are built
    on-chip from the kernel features.
  - ACT (ScalarE): every PSUM eviction fused with BN bias (+ReLU for dw),
    and the xcorr diag builds (Copy activation, per-partition scale=k2 col).
  - DVE: remaining xcorr taps as fp32 scalar_tensor_tensor; the first DVE
    tap reads the PE's PSUM partial directly as its accumulator seed.
  - GPSIMD: the ReLU6 min-6 clamps (1x-mode DVE never contends for the
    shared SBUF port pair).
Loop is software-pipelined: PE order per sample b is
  dw(b) -> xcorr(b-1) -> pw(b), hiding all eviction latencies.
"""

import numpy as np

import concourse.bass as bass
import concourse.mybir as mybir
from concourse.tile import TileContext
from concourse.bass_utils import run_bass_kernel_spmd

F32 = mybir.dt.float32
F32R = mybir.dt.float32r
F16 = mybir.dt.float16
AF = mybir.ActivationFunctionType
OP = mybir.AluOpType

B, C, KH, SH, KK = 128, 256, 7, 31, 3
N_CORES = 8
BPC = B // N_CORES          # samples per core
G = C // 128                # channel blocks
EPS = 1e-5

HO_K, HO_S, HO_X = KH - 2, SH - 2, 25   # 5, 29, 25

# --- tunables -------------------------------------------------------------
P_TAPS = 16                 # xcorr taps done on PE (of 25); rest on DVE
ACT_BUILDS = 22             # of the 2*P_TAPS diag builds, how many on ACT
                            # (rest go to GPSIMD)
# row split of the 25x25 xcorr output into two PSUM banks
XR_A = 13                   # rows 0..12  -> 325 cols
XR_B = HO_X - XR_A          # rows 13..24 -> 300 cols
# row split of the 29x29 dw/pw output into two PSUM banks
SR_A = 17                   # rows 0..16  -> 493 cols
SR_B = HO_S - SR_A          # rows 17..28 -> 348 cols

TAPS = [(t // 5, t % 5) for t in range(25)]
PE_TAPS = TAPS[:P_TAPS]
DVE_TAPS = TAPS[P_TAPS:]

# fp16 params column offsets
O_DWK = 0                       # 2g * 9taps * 128
O_DWS = O_DWK + G * 9 * 128
O_PWK = O_DWS + G * 9 * 128     # 4 blocks * 128
O_PWS = O_PWK + G * G * 128
O_I = O_PWS + G * G * 128       # identity 128
NP16 = O_I + 128
# fp32 params column offsets (biases + fp32 identity)
O_BDK, O_BDS, O_BPK, O_BPS = 0, G, 2 * G, 3 * G
O_I32 = 4 * G
NP32 = 4 * G + 128

_cache: dict = {}

LAST_RESULTS = None         # stash for test harness (exec_time_ns etc.)


def _fold_branch(dw_w, bn1, pw_w, pw_b, bn2):
    """Fold eval-mode BN params into conv weights/biases (host, numpy fp32)."""
    g1, b1, m1, v1 = bn1[0], bn1[1], bn1[2], bn1[3]
    inv1 = g1 / np.sqrt(v1 + EPS)
    shift1 = (b1 - m1 * inv1).astype(np.float32)
    dw = (dw_w[:, 0] * inv1[:, None, None]).reshape(C, 9).astype(np.float32)

    g2, b2, m2, v2 = bn2[0], bn2[1], bn2[2], bn2[3]
    inv2 = g2 / np.sqrt(v2 + EPS)
    shift2 = b2 - m2 * inv2
    W = (pw_w[:, :, 0, 0] * inv2[:, None]).astype(np.float32)   # (co, ci)
    bias2 = (pw_b * inv2 + shift2).astype(np.float32)

    # dw diag blocks: ddiag[g, t] = diag(dw[128g:128(g+1), t])  [ci, co]
    ddiag = np.zeros((G, 9, 128, 128), np.float32)
    for g in range(G):
        for t in range(9):
            np.fill_diagonal(ddiag[g, t], dw[128 * g:128 * (g + 1), t])

    # pw lhsT blocks for PE: lhsT[gi, go][ci, co] = W[go*128+co, gi*128+ci]
    lhsT = np.zeros((G, G, 128, 128), np.float32)
    for gi in range(G):
        for go in range(G):
            lhsT[gi, go] = W[go * 128:(go + 1) * 128, gi * 128:(gi + 1) * 128].T
    b1_blk = shift1.reshape(G, 128)
    b2_blk = bias2.reshape(G, 128)
    return ddiag, b1_blk, lhsT, b2_blk


def _split_waits(nc, keep=1):
    """This container's walrus accepts only one sync-wait per instruction.
    Move extra waits onto standalone EventSemaphore instructions placed just
    before the owning instruction in its engine stream (same semantics: the
    engine's sequencer stalls on each in turn)."""
    import bass_rust

    n = 0
    for bb in nc.m.functions[0].blocks:
        out = []
        for ins in bb.instructions:
            si = ins.sync_info
            if si is not None and len(si.on_wait) > keep:
                waits = list(si.on_wait)
                for w in waits[:-keep]:
                    n += 1
                    ev = mybir.InstEventSemaphore(
                        name=f"antsplitw_{n}", ins=[], outs=[])
                    ev.engine = ins.engine
                    ev.sync_info = bass_rust.SyncInfo(on_wait=[w], on_update=[])
                    out.append(ev)
                ins.sync_info = bass_rust.SyncInfo(
                    on_wait=waits[-keep:], on_update=list(si.on_update))
            out.append(ins)
        bb.instructions = out
    return n


def _build_nc():
    """Build the per-core Bass kernel (same program on all 8 cores)."""
    nc = bass.Bass()

    kern_h = nc.declare_dram_parameter("kern_in", [BPC, C, KH, KH], F16, isOutput=False)
    srch_h = nc.declare_dram_parameter("srch_in", [BPC, C, SH, SH], F16, isOutput=False)
    p16_h = nc.declare_dram_parameter("prm16", [128, NP16], F16, isOutput=False)
    p32_h = nc.declare_dram_parameter("prm32", [128, NP32], F32, isOutput=False)
    out_h = nc.declare_dram_parameter("out", [BPC, C, HO_X, HO_X], F32, isOutput=True)

    with TileContext(nc) as tc:
        with (
            tc.tile_pool(name="const", bufs=1) as cpool,
            tc.tile_pool(name="sio", bufs=3) as spool,
            tc.tile_pool(name="hbuf", bufs=2) as hpool,
            tc.tile_pool(name="s2buf", bufs=2) as s2pool,
            tc.tile_pool(name="diag", bufs=2) as dpool,
            tc.tile_pool(name="xout", bufs=2) as xpool,
            tc.tile_pool(name="ps", bufs=2, space="PSUM") as ppool,
        ):
            # ---- constants into SBUF ----
            p16 = cpool.tile([128, NP16], F16)
            nc.sync.dma_start(out=p16[:], in_=p16_h[:])
            p32 = cpool.tile([128, NP32], F32)
            nc.sync.dma_start(out=p32[:], in_=p32_h[:])

            def _dwd(base, g, t):     # dw diag lhsT [128,128] fp16
                o = base + (g * 9 + t) * 128
                return p16[:, o:o + 128]

            def _wp(base, gi, go):    # pw lhsT block [128,128] fp16
                o = base + (gi * G + go) * 128
                return p16[:, o:o + 128]

            def _eye():               # identity [128,128] fp16
                return p16[:, O_I:O_I + 128]

            def _eye32():             # identity [128,128] fp32
                return p32[:, O_I32:O_I32 + 128]

            def _b(base, g):          # bias col [128,1] fp32
                return p32[:, base + g:base + g + 1]

            # =========== Phase A: kernel branch, all 16 samples ===========
            hk = []
            for g in range(G):
                xk = cpool.tile([128, BPC, KH, KH], F16, name=f"xk{g}")
                for b in range(BPC):
                    nc.sync.dma_start(out=xk[:, b],
                                      in_=kern_h[b, 128 * g:128 * (g + 1)])
                psK = ppool.tile([128, BPC, HO_K, HO_K], F32, tag="psa",
                                 name="psK")
                for t in range(9):
                    u, v = t // 3, t % 3
                    nc.tensor.matmul(
                        psK[:], _dwd(O_DWK, g, t),
                        xk[:, :, u:u + HO_K, v:v + HO_K],
                        start=(t == 0), stop=(t == 8))
                h = cpool.tile([128, BPC, HO_K, HO_K], F16, name=f"hk{g}")
                nc.scalar.activation(h[:], psK[:], AF.Relu, bias=_b(O_BDK, g),
                                     scale=1.0)
                nc.vector.tensor_scalar(h[:], h[:], 6.0, None, OP.min)
                hk.append(h)
            K2 = []
            for go in range(G):
                psK2 = ppool.tile([128, BPC, HO_K, HO_K], F32, tag="psb",
                                  name="psK2")
                for gi in range(G):
                    nc.tensor.matmul(psK2[:], _wp(O_PWK, gi, go), hk[gi][:],
                                     start=(gi == 0), stop=(gi == G - 1))
                k2 = cpool.tile([128, BPC, HO_K * HO_K], F32, name=f"k2{go}")
                nc.scalar.activation(k2[:], psK2[:], AF.Identity,
                                     bias=_b(O_BPK, go), scale=1.0)
                K2.append(k2)

            # =========== Phase B: search branch + xcorr, pipelined ===========
            def load_xs(b):
                tiles = []
                for g in range(G):
                    xs = spool.tile([128, SH, SH], F16, tag=f"xs{g}",
                                    name=f"xs{g}")
                    nc.sync.dma_start(out=xs[:],
                                      in_=srch_h[b, 128 * g:128 * (g + 1)])
                    tiles.append(xs)
                return tiles

            def dw_search(b, xs_tiles):
                hs = []
                for g in range(G):
                    psa = ppool.tile([128, SR_A, HO_S], F32, tag="psa",
                                     name="psa")
                    psb = ppool.tile([128, SR_B, HO_S], F32, tag="psb",
                                     name="psb")
                    xs = xs_tiles[g]
                    for t in range(9):
                        u, v = t // 3, t % 3
                        nc.tensor.matmul(
                            psa[:], _dwd(O_DWS, g, t),
                            xs[:, u:u + SR_A, v:v + HO_S],
                            start=(t == 0), stop=(t == 8))
                        nc.tensor.matmul(
                            psb[:], _dwd(O_DWS, g, t),
                            xs[:, u + SR_A:u + HO_S, v:v + HO_S],
                            start=(t == 0), stop=(t == 8))
                    h = hpool.tile([128, HO_S * HO_S + 1], F16, tag=f"hs{g}",
                                   name=f"hs{g}")
                    na = SR_A * HO_S
                    nc.scalar.activation(h[:, 0:na], psa[:], AF.Relu,
                                         bias=_b(O_BDS, g), scale=1.0)
                    nc.scalar.activation(h[:, na:HO_S * HO_S], psb[:],
                                         AF.Relu, bias=_b(O_BDS, g), scale=1.0)
                    # min-6 clamp in place; 842 cols (incl 1 pad) keeps the
                    # inner dim even for the DVE 2-port fast mode
                    nc.vector.tensor_scalar(h[:], h[:], 6.0, None, OP.min)
                    hs.append(h)
                return hs

            def pw_search(b, hs):
                s2 = []
                na = SR_A * HO_S
                for go in range(G):
                    psa = ppool.tile([128, SR_A, HO_S], F32, tag="psa",
                                     name="ppa")
                    psb = ppool.tile([128, SR_B, HO_S], F32, tag="psb",
                                     name="ppb")
                    for gi in range(G):
                        nc.tensor.matmul(psa[:], _wp(O_PWS, gi, go),
                                         hs[gi][:, 0:na],
                                         start=(gi == 0), stop=(gi == G - 1))
                        nc.tensor.matmul(psb[:], _wp(O_PWS, gi, go),
                                         hs[gi][:, na:HO_S * HO_S],
                                         start=(gi == 0), stop=(gi == G - 1))
                    t = s2pool.tile([128, HO_S, HO_S], F16, tag=f"s2{go}",
                                    name=f"s2{go}")
                    nc.scalar.activation(t[:, 0:SR_A, :], psa[:], AF.Identity,
                                         bias=_b(O_BPS, go), scale=1.0)
                    nc.scalar.activation(t[:, SR_A:HO_S, :], psb[:],
                                         AF.Identity, bias=_b(O_BPS, go),
                                         scale=1.0)
                    s2.append(t)
                return s2

            def xcorr_dve_taps(b, s2):
                """DVE's share of the xcorr taps; independent of the PE part."""
                accs = []
                for g in range(G):
                    k2 = K2[g]
                    s2g = s2[g]
                    acc = xpool.tile([128, HO_X, HO_X], F16, tag=f"ax{g}",
                                     name=f"ax{g}")
                    (u0, v0) = DVE_TAPS[0]
                    t0 = u0 * 5 + v0
                    nc.vector.tensor_scalar(
                        acc[:], s2g[:, u0:u0 + HO_X, v0:v0 + HO_X],
                        k2[:, b, t0:t0 + 1], None, OP.mult)
                    for (u, v) in DVE_TAPS[1:]:
                        ti = u * 5 + v
                        nc.vector.scalar_tensor_tensor(
                            acc[:], s2g[:, u:u + HO_X, v:v + HO_X],
                            k2[:, b, ti:ti + 1], acc[:], OP.mult, OP.add)
                    accs.append(acc)
                return accs

            def xcorr_builds(b):
                """Diag matrices for the PE taps, split ACT / GPSIMD."""
                dalls = []
                for g in range(G):
                    k2 = K2[g]
                    dall = dpool.tile([128, P_TAPS * 128], F16, tag=f"da{g}",
                                      name=f"da{g}")
                    for i, (u, v) in enumerate(PE_TAPS):
                        ti = u * 5 + v
                        d = dall[:, i * 128:(i + 1) * 128]
                        if g * P_TAPS + i < ACT_BUILDS:
                            nc.scalar.activation(d, _eye(), AF.Copy,
                                                 bias=0.0,
                                                 scale=k2[:, b, ti:ti + 1])
                        else:
                            nc.vector.tensor_scalar(d, _eye(),
                                                    k2[:, b, ti:ti + 1],
                                                    None, OP.mult)
                    dalls.append(dall)
                return dalls

            def xcorr_pe(b, s2, dalls):
                """PE's taps into PSUM, then DVE combines into acc + DMA out."""
                for g in range(G):
                    s2g = s2[g]
                    dall = dalls[g]
                    pxa = ppool.tile([128, XR_A, HO_X], F32, tag="pxa",
                                     name="pxa")
                    pxb = ppool.tile([128, XR_B, HO_X], F32, tag="pxb",
                                     name="pxb")
                    n = len(PE_TAPS)
                    for i, (u, v) in enumerate(PE_TAPS):
                        d = dall[:, i * 128:(i + 1) * 128]
                        nc.tensor.matmul(
                            pxa[:], d,
                            s2g[:, u:u + XR_A, v:v + HO_X],
                            start=(i == 0), stop=(i == n - 1))
                        nc.tensor.matmul(
                            pxb[:], d,
                            s2g[:, u + XR_A:u + HO_X, v:v + HO_X],
                            start=(i == 0), stop=(i == n - 1))
                    na = XR_A * HO_X
                    xa16 = dpool.tile([128, HO_X * HO_X], F16, tag=f"xa{g}",
                                      name=f"xa{g}")
                    nc.scalar.activation(xa16[:, 0:na], pxa[:], AF.Identity,
                                         bias=0.0, scale=1.0)
                    nc.scalar.activation(xa16[:, na:HO_X * HO_X], pxb[:],
                                         AF.Identity, bias=0.0, scale=1.0)
                    acc = accs_prev[g]
                    out32 = xpool.tile([128, HO_X * HO_X], F32, tag=f"o{g}",
                                       name=f"o{g}")
                    nc.vector.tensor_tensor(
                        out32[:], acc[:].rearrange("p a b -> p (a b)"),
                        xa16[:], OP.add)
                    nc.sync.dma_start(out=out_h[b, 128 * g:128 * (g + 1)],
                                      in_=out32[:].rearrange(
                                          "p (a b) -> p a b", a=HO_X))

            xs_cur = load_xs(0)
            prev_s2 = None
            accs_prev = None
            for b in range(BPC):
                xs_next = load_xs(b + 1) if b + 1 < BPC else None
                if prev_s2 is not None:
                    accs_prev = xcorr_dve_taps(b - 1, prev_s2)
                    dalls = xcorr_builds(b - 1)
                hs = dw_search(b, xs_cur)
                if prev_s2 is not None:
                    xcorr_pe(b - 1, prev_s2, dalls)
                prev_s2 = pw_search(b, hs)
                xs_cur = xs_next
            accs_prev = xcorr_dve_taps(BPC - 1, prev_s2)
            dalls = xcorr_builds(BPC - 1)
            xcorr_pe(BPC - 1, prev_s2, dalls)

    _split_waits(nc)
    return nc


def kernel(kernel, search, k_dw_w, k_bn1, k_pw_w, k_pw_b, k_bn2,
           s_dw_w, s_bn1, s_pw_w, s_pw_b, s_bn2):
    global LAST_RESULTS
    kdd, kb1, kpw, kb2 = _fold_branch(np.asarray(k_dw_w), np.asarray(k_bn1),
                                      np.asarray(k_pw_w), np.asarray(k_pw_b),
                                      np.asarray(k_bn2))
    sdd, sb1, spw, sb2 = _fold_branch(np.asarray(s_dw_w), np.asarray(s_bn1),
                                      np.asarray(s_pw_w), np.asarray(s_pw_b),
                                      np.asarray(s_bn2))
    kern = np.ascontiguousarray(np.asarray(kernel, np.float16))
    srch = np.ascontiguousarray(np.asarray(search, np.float16))

    if "nc" not in _cache:
        _cache["nc"] = _build_nc()
    nc = _cache["nc"]

    prm16 = np.zeros((128, NP16), np.float16)
    # dw diags [g, t, ci, co] -> [ci, (g,t,co)]
    prm16[:, O_DWK:O_DWK + G * 9 * 128] = \
        kdd.transpose(2, 0, 1, 3).reshape(128, G * 9 * 128).astype(np.float16)
    prm16[:, O_DWS:O_DWS + G * 9 * 128] = \
        sdd.transpose(2, 0, 1, 3).reshape(128, G * 9 * 128).astype(np.float16)
    prm16[:, O_PWK:O_PWK + G * G * 128] = \
        kpw.transpose(2, 0, 1, 3).reshape(128, G * G * 128).astype(np.float16)
    prm16[:, O_PWS:O_PWS + G * G * 128] = \
        spw.transpose(2, 0, 1, 3).reshape(128, G * G * 128).astype(np.float16)
    prm16[:, O_I:O_I + 128] = np.eye(128, dtype=np.float16)

    prm32 = np.zeros((128, NP32), np.float32)
    prm32[:, O_BDK:O_BDK + G] = kb1.T
    prm32[:, O_BDS:O_BDS + G] = sb1.T
    prm32[:, O_BPK:O_BPK + G] = kb2.T
    prm32[:, O_BPS:O_BPS + G] = sb2.T
    prm32[:, O_I32:O_I32 + 128] = np.eye(128, dtype=np.float32)

    in_maps = []
    for i in range(N_CORES):
        sl = slice(i * BPC, (i + 1) * BPC)
        in_maps.append({"kern_in": kern[sl], "srch_in": srch[sl],
                        "prm16": prm16, "prm32": prm32})

    res = run_bass_kernel_spmd(nc, in_maps, list(range(N_CORES)))
    LAST_RESULTS = res
    out = np.concatenate([res.results[i]["out"] for i in range(N_CORES)], axis=0)
    return out
